# revision 6
# baseline (speedup 1.0000x reference)
"""Trainium2 Bass kernel for nn_CATS_Attention.

Data-parallel over the batch dim: 1024 batches -> 8 NeuronCores x 128.
Per core, per batch m:
  h1 = tanh(Wa @ [Xq_m; Xp1_m])  (128k x 128n), s1 = va @ h1  (scores)
  beta1 = softmax(s1)            (no max-subtraction: |s| < ~3)
  p1 = Xp1_m @ beta1             (768,)   (same for pool 2)
  z* = relu(W21 @ {p1, p2, qv} + b21)  with W21 = W2 @ W1 (host-fused)
  o  = relu(w3 . [z1, z2, |z1-z2|, |z1-zq|, |z2-zq|] + b3)

X is shipped ONCE, in score layout only (e on partitions, (chunk, batch, n)
on the free axis) -- ~75.5 MB/core instead of the 126 MB dual-layout
shipment, putting the kernel at the HBM roofline.  Pooling is computed
on-chip from the same tiles:

  - the score row s[n] is broadcast to all 128 partitions by a single PE
    matmul whose stationary operand is va replicated to 128 columns; exp on
    the Scalar engine then gives E (unnormalized softmax) broadcast across
    partitions as a [128, 4b*128n] tile.
  - DVE multiplies X chunks by E (2x-mode bf16 tensor_tensor with a
    stride-0 broadcast over the 6 e-chunks), GpSimd folds n 128->16 with
    fp32 tree adds, and a final DVE tensor_reduce produces the pooled
    p[e-chunk, batch] -- already in the layout the MLP matmuls consume
    (no DRAM-roundtrip transposes).
  - normalization by 1/sum(E) is folded into a single whole-core
    tensor_tensor at the end.

The shared Wa_q @ Xq part of both score matmuls is computed once per quad;
GpSimd adds it to each pool's Wa_p @ Xp PSUM bank before tanh.
All activation functions used (tanh/exp/relu/abs) live in the single
`exp_and_others` hardware table, so there are no table reloads.
"""

import os
import sys

import numpy as np

for _p in ("/opt/trn_rl_repo", "/root/.axon_site/_ro/trn_rl_repo"):
    if os.path.isdir(_p) and _p not in sys.path:
        sys.path.insert(0, _p)

import ml_dtypes

BF16 = ml_dtypes.bfloat16

EMB = 768
SEQ = 128          # n (attention positions) == attention dim k
M_TOTAL = 1024
N_CORES = 8
M_PER_CORE = M_TOTAL // N_CORES   # 128
NQUAD = M_PER_CORE // 4           # 32 quads of 4 batches
NCH = EMB // 128                  # 6 chunks of the embedding dim

_PROGRAM_CACHE = {}


def _build_program(nquad):
    """One Bass program, SPMD across cores (inputs differ per core)."""
    import concourse.bass as bass
    import concourse.tile as tile
    from concourse import bacc, mybir

    dt = mybir.dt
    AF = mybir.ActivationFunctionType
    Alu = mybir.AluOpType
    AX = mybir.AxisListType

    nb = 4 * nquad  # batches per core
    nc = bacc.Bacc(None, target_bir_lowering=False, debug=False)

    # ---- per-core parameters (host pre-permuted, see kernel() below) ----
    # xn[q][p][c][b][n]: score layout, 18 chunks (q 0-5, p1 6-11, p2 12-17)
    xn = nc.declare_dram_parameter(
        "xn", [nquad, 128, 18 * 512], dt.bfloat16, isOutput=False
    )
    # qv[p][c][m]: query vectors as (e_chunk, batch) columns
    qv = nc.declare_dram_parameter("qv", [128, NCH * nb], dt.bfloat16, isOutput=False)
    # wat[p][c][k]: Wa.T chunks (12 of them)
    wat = nc.declare_dram_parameter("wat", [128, 12 * 128], dt.bfloat16, isOutput=False)
    # va replicated to 128 columns: stationary for the score-broadcast matmul
    va_rep = nc.declare_dram_parameter("va_rep", [128, 128], dt.bfloat16, isOutput=False)
    # w21t[p][ei][eo][col]: (W2@W1).T chunk grid
    w21t = nc.declare_dram_parameter("w21t", [128, NCH * NCH * 128], dt.bfloat16, isOutput=False)
    # w3[p][s*6+c]: W3 column chunks for the 5 z-segments
    w3 = nc.declare_dram_parameter("w3", [128, 5 * NCH], dt.bfloat16, isOutput=False)
    b21 = nc.declare_dram_parameter("b21", [128, NCH], dt.float32, isOutput=False)
    b3 = nc.declare_dram_parameter("b3", [1, 1], dt.float32, isOutput=False)
    out = nc.declare_dram_parameter("out", [1, nb], dt.float32, isOutput=True)

    with tile.TileContext(nc) as tc:
        from contextlib import ExitStack

        with ExitStack() as ctx:
            const_pool = ctx.enter_context(tc.tile_pool(name="const", bufs=1))
            xn_pool = ctx.enter_context(tc.tile_pool(name="xn_p", bufs=3))
            h_pool = ctx.enter_context(tc.tile_pool(name="h_p", bufs=2))
            e_pool = ctx.enter_context(tc.tile_pool(name="e_p", bufs=2))
            prod_pool = ctx.enter_context(tc.tile_pool(name="prod_p", bufs=2))
            tree_pool = ctx.enter_context(tc.tile_pool(name="tree_p", bufs=2))
            acc_pool = ctx.enter_context(tc.tile_pool(name="acc_p", bufs=1))
            mlp_pool = ctx.enter_context(tc.tile_pool(name="mlp_p", bufs=1))
            psum_pool = ctx.enter_context(
                tc.tile_pool(name="psum", bufs=1, space="PSUM")
            )

            # ---- persistent constants ----
            # wat + va_rep ride the SP queue ahead of the xn stream so the
            # first score matmul isn't blocked behind other consts
            wat_sb = const_pool.tile([128, 12 * 128], dt.bfloat16)
            nc.sync.dma_start(wat_sb[:], wat[:])
            va_sb = const_pool.tile([128, 128], dt.bfloat16)
            nc.sync.dma_start(va_sb[:], va_rep[:])
            # MLP-phase constants on the scalar HWDGE queue
            w21t_sb = const_pool.tile([128, NCH * NCH * 128], dt.bfloat16)
            nc.scalar.dma_start(w21t_sb[:], w21t[:])
            w3_sb = const_pool.tile([128, 5 * NCH], dt.bfloat16)
            nc.scalar.dma_start(w3_sb[:], w3[:])
            b21_sb = const_pool.tile([128, NCH], dt.float32)
            nc.scalar.dma_start(b21_sb[:], b21[:])
            b3_sb = const_pool.tile([1, 1], dt.float32)
            nc.scalar.dma_start(b3_sb[:], b3[:])
            qv_sb = const_pool.tile([128, NCH * nb], dt.bfloat16)
            nc.scalar.dma_start(qv_sb[:], qv[:])

            # pooled rows (raw, un-normalized) [p][pool][c][m] and exp-sums
            p_all = acc_pool.tile([128, 2 * NCH * nb], dt.float32)
            esum = acc_pool.tile([128, 2 * nb], dt.float32)

            for q in range(nquad):
                t_q = xn_pool.tile([128, 18 * 512], dt.bfloat16, name="t_q")
                nc.sync.dma_start(t_q[:], xn[q])

                # shared q-part: sq = Wa_q @ Xq  (kept in PSUM for both pools)
                psq = psum_pool.tile(
                    [128, 512], dt.float32, tag="sq", bufs=2, name="psq"
                )
                for c in range(6):
                    nc.tensor.matmul(
                        psq[:], wat_sb[:, c * 128:(c + 1) * 128],
                        t_q[:, c * 512:(c + 1) * 512],
                        start=(c == 0), stop=(c == 5),
                    )
                # pool parts: sp_i = Wa_p @ Xp_i, interleaved so each Wa_p
                # chunk is loaded into the PE stationary registers once
                psp1 = psum_pool.tile(
                    [128, 512], dt.float32, tag="sp1", bufs=2, name="psp1"
                )
                psp2 = psum_pool.tile(
                    [128, 512], dt.float32, tag="sp2", bufs=2, name="psp2"
                )
                for c in range(6):
                    st = wat_sb[:, (6 + c) * 128:(7 + c) * 128]
                    nc.tensor.matmul(
                        psp1[:], st, t_q[:, (6 + c) * 512:(7 + c) * 512],
                        start=(c == 0), stop=(c == 5),
                    )
                    nc.tensor.matmul(
                        psp2[:], st, t_q[:, (12 + c) * 512:(13 + c) * 512],
                        start=(c == 0), stop=(c == 5),
                    )

                # DVE may read only one PSUM operand per op, so the shared
                # q-part is staged to SBUF by the (mostly idle) Scalar engine
                sq_sb = h_pool.tile([128, 512], dt.float32, tag="sqsb", name="sq_sb")
                nc.scalar.copy(sq_sb[:], psq[:])

                for pool_i, psp in ((0, psp1), (1, psp2)):
                    # h = tanh(sq + sp); the add runs on DVE so the PE
                    # doesn't re-accumulate the shared q-part
                    hpre = h_pool.tile(
                        [128, 512], dt.float32, tag=f"hpre{pool_i}", name="hpre"
                    )
                    nc.vector.tensor_tensor(hpre[:], psp[:], sq_sb[:], Alu.add)
                    h_sb = h_pool.tile(
                        [128, 512], dt.bfloat16, tag=f"h{pool_i}", name="h_sb"
                    )
                    nc.scalar.activation(h_sb[:], hpre[:], AF.Tanh)

                    # score rows broadcast to every partition: one matmul with
                    # va replicated as the stationary -> S[p, (b, n)] = s_b[n]
                    pS = psum_pool.tile(
                        [128, 512], dt.float32, tag=f"S{pool_i}", name="pS"
                    )
                    nc.tensor.matmul(pS[:], va_sb[:], h_sb[:], start=True, stop=True)
                    E = e_pool.tile(
                        [128, 512], dt.bfloat16, tag=f"E{pool_i}", name="E"
                    )
                    nc.scalar.activation(E[:], pS[:], AF.Exp)

                    # product P[e, (c, b, n)] = X * E (DVE 2x, E broadcast
                    # over the 6 e-chunks via a stride-0 free dim)
                    prod = prod_pool.tile(
                        [128, 6 * 512], dt.bfloat16, tag=f"pr{pool_i}", name="prod"
                    )
                    x_sl = t_q[:, (6 + 6 * pool_i) * 512:(12 + 6 * pool_i) * 512]
                    nc.vector.tensor_tensor(
                        prod[:].rearrange("p (c j) -> p c j", c=6),
                        x_sl.rearrange("p (c j) -> p c j", c=6),
                        E[:].rearrange("p (one j) -> p one j", one=1)
                            .broadcast_to((128, 6, 512)),
                        Alu.mult,
                    )
                    # fold n: 128 -> 16 with fp32 tree adds on GpSimd
                    pr = prod[:].rearrange("p (k n) -> p k n", n=128)
                    s1 = tree_pool.tile(
                        [128, 24 * 64], dt.float32, tag=f"s1{pool_i}", name="s1"
                    )
                    s1r = s1[:].rearrange("p (k n) -> p k n", n=64)
                    nc.gpsimd.tensor_tensor(
                        s1r, pr[:, :, 0:64], pr[:, :, 64:128], Alu.add
                    )
                    s2 = tree_pool.tile(
                        [128, 24 * 32], dt.float32, tag=f"s2{pool_i}", name="s2"
                    )
                    s2r = s2[:].rearrange("p (k n) -> p k n", n=32)
                    nc.gpsimd.tensor_tensor(
                        s2r, s1r[:, :, 0:32], s1r[:, :, 32:64], Alu.add
                    )
                    s3 = tree_pool.tile(
                        [128, 24 * 16], dt.float32, tag=f"s3{pool_i}", name="s3"
                    )
                    s3r = s3[:].rearrange("p (k n) -> p k n", n=16)
                    nc.gpsimd.tensor_tensor(
                        s3r, s2r[:, :, 0:16], s2r[:, :, 16:32], Alu.add
                    )
                    # final fold 16 -> 1 on DVE, straight into the MLP layout
                    p_dst = p_all[:].rearrange("p (s m) -> p s m", m=nb)[
                        :, pool_i * 6:(pool_i + 1) * 6, q * 4:q * 4 + 4
                    ]
                    nc.vector.tensor_reduce(p_dst, s3r, AX.X, Alu.add)
                    # sum(E) per batch (replicated on all partitions, which
                    # the final normalizing multiply wants anyway)
                    e_dst = esum[:].rearrange("p (s m) -> p s m", m=nb)[
                        :, pool_i, q * 4:q * 4 + 4
                    ]
                    nc.vector.tensor_reduce(
                        e_dst, E[:].rearrange("p (b n) -> p b n", n=128),
                        AX.X, Alu.add,
                    )

            # ---- normalize: p = p_raw / sum(E), one whole-core multiply ----
            recip = acc_pool.tile([128, 2 * nb], dt.float32)
            nc.vector.reciprocal(recip[:], esum[:])
            p_bf = acc_pool.tile([128, 2 * NCH * nb], dt.bfloat16)
            nc.vector.tensor_tensor(
                p_bf[:].rearrange("p (pl c m) -> p pl c m", pl=2, c=NCH),
                p_all[:].rearrange("p (pl c m) -> p pl c m", pl=2, c=NCH),
                recip[:].rearrange("p (pl one m) -> p pl one m", pl=2, one=1)
                    .broadcast_to((128, 2, NCH, nb)),
                Alu.mult,
            )

            # ---- MLP + head ----
            rhs_of = {
                0: lambda ei: p_bf[:, ei * nb:(ei + 1) * nb],
                1: lambda ei: p_bf[:, (NCH + ei) * nb:(NCH + ei + 1) * nb],
                2: lambda ei: qv_sb[:, ei * nb:(ei + 1) * nb],
            }
            z_tiles = []
            for inp_i in range(3):
                z_sb = mlp_pool.tile([128, NCH * nb], dt.bfloat16, name=f"z{inp_i}")
                for eo in range(NCH):
                    # reuses the score-phase sq banks (free by the MLP phase)
                    pz = psum_pool.tile(
                        [128, nb], dt.float32, tag="sq", bufs=2, name="pz"
                    )
                    for ei in range(NCH):
                        nc.tensor.matmul(
                            pz[:],
                            w21t_sb[:, (ei * NCH + eo) * 128:(ei * NCH + eo + 1) * 128],
                            rhs_of[inp_i](ei),
                            start=(ei == 0), stop=(ei == NCH - 1),
                        )
                    nc.scalar.activation(
                        z_sb[:, eo * nb:(eo + 1) * nb], pz[:], AF.Relu,
                        bias=b21_sb[:, eo:eo + 1],
                    )
                z_tiles.append(z_sb)
            z1, z2, zq = z_tiles
            d_tiles = []
            for di, (a_t, b_t) in enumerate(((z1, z2), (z1, zq), (z2, zq))):
                d_f = mlp_pool.tile(
                    [128, NCH * nb], dt.float32, tag="d_f", bufs=2, name="d_f"
                )
                nc.vector.tensor_sub(d_f[:], a_t[:], b_t[:])
                d_b = mlp_pool.tile([128, NCH * nb], dt.bfloat16, name=f"d{di}")
                nc.scalar.activation(d_b[:], d_f[:], AF.Abs)
                d_tiles.append(d_b)
            po = psum_pool.tile([1, nb], dt.float32, tag="S0", name="po")
            k = 0
            nmm = 5 * NCH
            for s, zt in enumerate([z1, z2] + d_tiles):
                for c in range(NCH):
                    nc.tensor.matmul(
                        po[0:1, :], w3_sb[:, s * NCH + c:s * NCH + c + 1],
                        zt[:, c * nb:(c + 1) * nb],
                        start=(k == 0), stop=(k == nmm - 1),
                    )
                    k += 1
            o_sb = mlp_pool.tile([1, nb], dt.float32)
            nc.scalar.activation(o_sb[:], po[:], AF.Relu, bias=b3_sb[0:1, 0:1])
            nc.sync.dma_start(out[:], o_sb[:])

    nc.compile()
    return nc


def _get_program(nquad):
    if nquad not in _PROGRAM_CACHE:
        _PROGRAM_CACHE[nquad] = _build_program(nquad)
    return _PROGRAM_CACHE[nquad]


def _prep_core_inputs(Xc):
    """Per-core X slice (mc, 2306, 128) fp32/bf16 -> DMA-ready arrays."""
    mc = Xc.shape[0]
    nquad = mc // 4
    X16 = Xc if Xc.dtype == BF16 else Xc.astype(BF16)
    rows = np.r_[0:2 * EMB, 2 * EMB + 1:3 * EMB + 1]
    xn = X16[:, rows, :]                                   # (mc, 2304, 128)
    xn = xn.reshape(nquad, 4, 18, 128, 128)                # q b c p n
    xn = np.ascontiguousarray(xn.transpose(0, 3, 2, 1, 4)) # q p c b n
    xn = xn.reshape(nquad, 128, 18 * 512)

    qvv = np.ascontiguousarray(X16[:, 0:EMB, 0].T)         # (768, mc)
    qvv = qvv.reshape(NCH, 128, mc)                        # c p m
    qvv = np.ascontiguousarray(qvv.transpose(1, 0, 2)).reshape(128, NCH * mc)
    return xn, qvv


def _prep_weights(Wa, va, W1, b1, W2, b2, W3, b3):
    wat = Wa.T.astype(np.float32)                          # (1536, 128)
    wat = wat.reshape(12, 128, 128).transpose(1, 0, 2)     # p c k
    wat = np.ascontiguousarray(wat).reshape(128, 12 * 128).astype(BF16)

    va_rep = np.ascontiguousarray(
        np.repeat(va[0][:, None], 128, axis=1)
    ).astype(BF16)                                         # (128, 128)

    W21 = (W2.astype(np.float32) @ W1.astype(np.float32))
    w21t = W21.T.reshape(NCH, 128, NCH, 128).transpose(1, 0, 2, 3)  # p ei eo c
    w21t = np.ascontiguousarray(w21t).reshape(128, NCH * NCH * 128).astype(BF16)

    w3 = W3[0].astype(np.float32).reshape(5, NCH, 128).transpose(2, 0, 1)  # p s c
    w3 = np.ascontiguousarray(w3).reshape(128, 5 * NCH).astype(BF16)

    b21 = (W2.astype(np.float32) @ b1.astype(np.float32) + b2.astype(np.float32))
    b21 = np.ascontiguousarray(b21.reshape(NCH, 128).T).astype(np.float32)  # (128, 6)
    b3a = np.array(b3, np.float32).reshape(1, 1)
    return dict(wat=wat, va_rep=va_rep, w21t=w21t, w3=w3, b21=b21, b3=b3a)


def kernel(X, Wa, va, W1, b1, W2, b2, W3, b3):
    from concourse.bass_utils import run_bass_kernel_spmd

    X = np.asarray(X)
    w = _prep_weights(
        np.asarray(Wa), np.asarray(va), np.asarray(W1), np.asarray(b1),
        np.asarray(W2), np.asarray(b2), np.asarray(W3), np.asarray(b3),
    )
    mc = X.shape[0] // N_CORES
    in_maps = []
    for c in range(N_CORES):
        xn, qvv = _prep_core_inputs(X[c * mc:(c + 1) * mc])
        in_maps.append(dict(xn=xn, qv=qvv, **w))
    nc = _get_program(mc // 4)
    res = run_bass_kernel_spmd(nc, in_maps, list(range(N_CORES)))
    out = np.concatenate(
        [res.results[i]["out"].reshape(-1) for i in range(N_CORES)]
    )
    return out.astype(np.float32)


if __name__ == "__main__":
    # smoke-build
    nc = _build_program(NQUAD)
    print("program built ok")


# revision 7
# speedup vs baseline: 1.4600x; 1.4600x over previous
"""Trainium2 Bass kernel for nn_CATS_Attention.

Data-parallel over the batch dim: 1024 batches -> 8 NeuronCores x 128.
Per core, per batch m:
  h1 = tanh(Wa @ [Xq_m; Xp1_m])  (128k x 128n), s1 = va @ h1  (scores)
  beta1 = softmax(s1)            (no max-subtraction: |s| < ~3)
  p1 = Xp1_m @ beta1             (768,)   (same for pool 2)
  z* = relu(W21 @ {p1, p2, qv} + b21)  with W21 = W2 @ W1 (host-fused)
  o  = relu(w3 . [z1, z2, |z1-z2|, |z1-zq|, |z2-zq|] + b3)

HBM traffic is 101 MB/core instead of the 126 MB dual-layout baseline:
X ships once in score layout (xn: e on partitions) plus a transposed copy
for POOL 1 ONLY (xpt1: n on partitions).  The two pools then use different
pooling engines so no single engine saturates:

  - pool 1: the PE pools via the classic trick -- exp(score^T) columns as
    a 32-col broadcast stationary against X^T, with a ones column giving
    sum(E); normalized on DVE (cheap: one reciprocal + 2 tensor_scalars
    per batch) and transposed back through a tiny DRAM roundtrip.
  - pool 2: pooled on-chip from the score-layout tiles.  The score row is
    broadcast to all partitions by one PE matmul (va replicated to 128
    stationary columns); exp on Scalar gives E broadcast as [128, 4b*128n];
    DVE forms X*E (2x-mode bf16 tensor_tensor, E broadcast over the 6
    e-chunks via a stride-0 free dim) and reduces n with one 1x
    tensor_reduce straight into the MLP's [e-part, batch-col] layout.
    Normalization by 1/sum(E) is one whole-core multiply at the end.

The shared Wa_q @ Xq part of both score matmuls is computed once per quad
(Scalar stages it to SBUF, DVE adds it to each pool's Wa_p @ Xp bank).
All activation functions used (tanh/exp/relu/abs/copy) live in the single
`exp_and_others` hardware table, so there are no table reloads.
"""

import os
import sys

import numpy as np

for _p in ("/opt/trn_rl_repo", "/root/.axon_site/_ro/trn_rl_repo"):
    if os.path.isdir(_p) and _p not in sys.path:
        sys.path.insert(0, _p)

import ml_dtypes

BF16 = ml_dtypes.bfloat16

EMB = 768
SEQ = 128          # n (attention positions) == attention dim k
M_TOTAL = 1024
N_CORES = 8
M_PER_CORE = M_TOTAL // N_CORES   # 128
NQUAD = M_PER_CORE // 4           # 32 quads of 4 batches
NCH = EMB // 128                  # 6 chunks of the embedding dim

_PROGRAM_CACHE = {}


def _build_program(nquad):
    """One Bass program, SPMD across cores (inputs differ per core)."""
    import concourse.bass as bass
    import concourse.tile as tile
    from concourse import bacc, mybir

    dt = mybir.dt
    AF = mybir.ActivationFunctionType
    Alu = mybir.AluOpType
    AX = mybir.AxisListType

    nb = 4 * nquad  # batches per core
    nc = bacc.Bacc(None, target_bir_lowering=False, debug=False)

    # ---- per-core parameters (host pre-permuted, see kernel() below) ----
    # xn[q][p][c][b][n]: score layout, 18 chunks (q 0-5, p1 6-11, p2 12-17)
    xn = nc.declare_dram_parameter(
        "xn", [nquad, 128, 18 * 512], dt.bfloat16, isOutput=False
    )
    # xpt1[q][p=n][b][e]: transposed Xp1 for the PE pooling of pool 1
    xpt1 = nc.declare_dram_parameter(
        "xpt1", [nquad, 128, 4 * EMB], dt.bfloat16, isOutput=False
    )
    # qv[p][c][m]: query vectors as (e_chunk, batch) columns
    qv = nc.declare_dram_parameter("qv", [128, NCH * nb], dt.bfloat16, isOutput=False)
    # wat[p][c][k]: Wa.T chunks (12 of them)
    wat = nc.declare_dram_parameter("wat", [128, 12 * 128], dt.bfloat16, isOutput=False)
    # va replicated to 128 columns: stationary for the pool-2 score-broadcast
    # matmul; column 0 doubles as the va moving column for pool-1 scores
    va_rep = nc.declare_dram_parameter("va_rep", [128, 128], dt.bfloat16, isOutput=False)
    # w21t[p][ei][eo][col]: (W2@W1).T chunk grid
    w21t = nc.declare_dram_parameter("w21t", [128, NCH * NCH * 128], dt.bfloat16, isOutput=False)
    # w3[p][s*6+c]: W3 column chunks for the 5 z-segments
    w3 = nc.declare_dram_parameter("w3", [128, 5 * NCH], dt.bfloat16, isOutput=False)
    b21 = nc.declare_dram_parameter("b21", [128, NCH], dt.float32, isOutput=False)
    b3 = nc.declare_dram_parameter("b3", [1, 1], dt.float32, isOutput=False)
    out = nc.declare_dram_parameter("out", [1, nb], dt.float32, isOutput=True)

    with tile.TileContext(nc) as tc:
        from contextlib import ExitStack

        with ExitStack() as ctx:
            const_pool = ctx.enter_context(tc.tile_pool(name="const", bufs=1))
            xn_pool = ctx.enter_context(tc.tile_pool(name="xn_p", bufs=3))
            xpt_pool = ctx.enter_context(tc.tile_pool(name="xpt_p", bufs=3))
            h_pool = ctx.enter_context(tc.tile_pool(name="h_p", bufs=2))
            e_pool = ctx.enter_context(tc.tile_pool(name="e_p", bufs=2))
            prod_pool = ctx.enter_context(tc.tile_pool(name="prod_p", bufs=2))
            r_pool = ctx.enter_context(tc.tile_pool(name="r_p", bufs=2))
            psb_pool = ctx.enter_context(tc.tile_pool(name="psb_p", bufs=2))
            acc_pool = ctx.enter_context(tc.tile_pool(name="acc_p", bufs=1))
            mlp_pool = ctx.enter_context(tc.tile_pool(name="mlp_p", bufs=1))
            psum_pool = ctx.enter_context(
                tc.tile_pool(name="psum", bufs=1, space="PSUM")
            )
            dram_pool = ctx.enter_context(
                tc.tile_pool(name="dram", bufs=1, space="DRAM")
            )

            # ---- persistent constants ----
            # wat + va ride the SP queue ahead of the xn stream
            wat_sb = const_pool.tile([128, 12 * 128], dt.bfloat16)
            nc.sync.dma_start(wat_sb[:], wat[:])
            va_sb = const_pool.tile([128, 128], dt.bfloat16)
            nc.sync.dma_start(va_sb[:], va_rep[:])
            # MLP-phase constants on the scalar HWDGE queue
            w21t_sb = const_pool.tile([128, NCH * NCH * 128], dt.bfloat16)
            nc.scalar.dma_start(w21t_sb[:], w21t[:])
            w3_sb = const_pool.tile([128, 5 * NCH], dt.bfloat16)
            nc.scalar.dma_start(w3_sb[:], w3[:])
            b21_sb = const_pool.tile([128, NCH], dt.float32)
            nc.scalar.dma_start(b21_sb[:], b21[:])
            b3_sb = const_pool.tile([1, 1], dt.float32)
            nc.scalar.dma_start(b3_sb[:], b3[:])
            qv_sb = const_pool.tile([128, NCH * nb], dt.bfloat16)
            nc.scalar.dma_start(qv_sb[:], qv[:])
            ones_sb = const_pool.tile([128, 1], dt.bfloat16)
            nc.vector.memset(ones_sb[:], 1.0)

            # pool-2 raw pooled rows [p][c][m] and exp-sums (all partitions)
            p2_all = acc_pool.tile([128, NCH * nb], dt.float32)
            esum2 = acc_pool.tile([128, nb], dt.float32)
            # pool-1 DRAM scratch for the pooled-row transpose roundtrip
            p_d1 = dram_pool.tile([nb, EMB], dt.bfloat16)

            for q in range(nquad):
                t_q = xn_pool.tile([128, 18 * 512], dt.bfloat16, name="t_q")
                nc.sync.dma_start(t_q[:], xn[q])
                # pool-1 transposed tiles ride the SWDGE queue (GpSimd is
                # otherwise idle) so they can't head-of-line block xn
                xt_q = xpt_pool.tile([128, 4 * EMB], dt.bfloat16, name="xt_q")
                nc.gpsimd.dma_start(xt_q[:], xpt1[q])

                # shared q-part: sq = Wa_q @ Xq, staged to SBUF by Scalar
                psq = psum_pool.tile(
                    [128, 512], dt.float32, tag="sq", name="psq"
                )
                for c in range(6):
                    nc.tensor.matmul(
                        psq[:], wat_sb[:, c * 128:(c + 1) * 128],
                        t_q[:, c * 512:(c + 1) * 512],
                        start=(c == 0), stop=(c == 5),
                    )
                sq_sb = h_pool.tile([128, 512], dt.float32, tag="sqsb", name="sq_sb")
                nc.scalar.copy(sq_sb[:], psq[:])

                # pool parts: sp_i = Wa_p @ Xp_i, interleaved so each Wa_p
                # chunk is loaded into the PE stationary registers once
                psp1 = psum_pool.tile(
                    [128, 512], dt.float32, tag="sp1", bufs=2, name="psp1"
                )
                psp2 = psum_pool.tile(
                    [128, 512], dt.float32, tag="sp2", name="psp2"
                )
                for c in range(6):
                    st = wat_sb[:, (6 + c) * 128:(7 + c) * 128]
                    nc.tensor.matmul(
                        psp1[:], st, t_q[:, (6 + c) * 512:(7 + c) * 512],
                        start=(c == 0), stop=(c == 5),
                    )
                    nc.tensor.matmul(
                        psp2[:], st, t_q[:, (12 + c) * 512:(13 + c) * 512],
                        start=(c == 0), stop=(c == 5),
                    )
                h_tiles = []
                for pool_i, psp in ((0, psp1), (1, psp2)):
                    hpre = h_pool.tile(
                        [128, 512], dt.float32, tag=f"hpre{pool_i}", name="hpre"
                    )
                    nc.vector.tensor_tensor(hpre[:], psp[:], sq_sb[:], Alu.add)
                    h_sb = h_pool.tile(
                        [128, 512], dt.bfloat16, tag=f"h{pool_i}", name="h_sb"
                    )
                    nc.scalar.activation(h_sb[:], hpre[:], AF.Tanh)
                    h_tiles.append(h_sb)

                # ---- pool 1: PE pooling from the transposed tiles ----
                # score columns sT[n, b] via per-batch h-slice stationaries
                ps1 = psum_pool.tile([128, 4], dt.float32, tag="ps1", name="ps1")
                for b in range(4):
                    nc.tensor.matmul(
                        ps1[:, b:b + 1],
                        h_tiles[0][:, b * 128:(b + 1) * 128], va_sb[:, 0:1],
                        start=True, stop=True,
                    )
                et = e_pool.tile([128, 4], dt.bfloat16, tag="et", name="et")
                nc.scalar.activation(et[:], ps1[:], AF.Exp)
                pa = psum_pool.tile([128, 512], dt.float32, tag="pa", name="pa")
                pb = psum_pool.tile([128, 257], dt.float32, tag="pb", name="pb")
                for b in range(4):
                    # E^T column broadcast to M=32 fills a whole 32-partition
                    # PSUM strip; the ones column appends sum(E)
                    lhs = et[:, b:b + 1].rearrange(
                        "p (m one) -> p m one", one=1
                    ).broadcast_to((128, 1, 32))
                    tp = (0, 32 * b)
                    sl = slice(32 * b, 32 * b + 32)
                    nc.tensor.matmul(
                        pa[sl, :], lhs, xt_q[:, b * EMB:b * EMB + 512],
                        start=True, stop=True, tile_position=tp,
                    )
                    nc.tensor.matmul(
                        pb[sl, 0:256], lhs,
                        xt_q[:, b * EMB + 512:b * EMB + 768],
                        start=True, stop=False, tile_position=tp,
                    )
                    nc.tensor.matmul(
                        pb[sl, 256:257], lhs, ones_sb[:, 0:1],
                        start=False, stop=True, tile_position=tp,
                    )
                psb = psb_pool.tile([128, EMB], dt.bfloat16, name="psb")
                r_sb = r_pool.tile([128, 1], dt.float32, name="r_sb")
                nc.vector.reciprocal(r_sb[:], pb[:, 256:257])
                nc.vector.tensor_scalar_mul(psb[:, 0:512], pa[:], r_sb[:])
                nc.vector.tensor_scalar_mul(
                    psb[:, 512:768], pb[:, 0:256], r_sb[:]
                )
                nc.scalar.dma_start(
                    p_d1[q * 4:(q + 1) * 4, :],
                    psb[0:128:32, :],
                )

                # ---- pool 2: on-chip pooling from the score-layout tile ----
                pS = psum_pool.tile([128, 512], dt.float32, tag="S2", name="pS")
                nc.tensor.matmul(pS[:], va_sb[:], h_tiles[1][:], start=True, stop=True)
                E = e_pool.tile([128, 512], dt.bfloat16, tag="E2", name="E")
                nc.scalar.activation(E[:], pS[:], AF.Exp)
                prod = prod_pool.tile(
                    [128, 6 * 512], dt.bfloat16, tag="pr2", name="prod"
                )
                nc.vector.tensor_tensor(
                    prod[:].rearrange("p (c j) -> p c j", c=6),
                    t_q[:, 12 * 512:18 * 512].rearrange("p (c j) -> p c j", c=6),
                    E[:].rearrange("p (one j) -> p one j", one=1)
                        .broadcast_to((128, 6, 512)),
                    Alu.mult,
                )
                p_dst = p2_all[:].rearrange("p (s m) -> p s m", m=nb)[
                    :, :, q * 4:q * 4 + 4
                ]
                nc.vector.tensor_reduce(
                    p_dst, prod[:].rearrange("p (k n) -> p k n", n=128),
                    AX.X, Alu.add,
                )
                e_dst = esum2[:, q * 4:q * 4 + 4]
                nc.vector.tensor_reduce(
                    e_dst, E[:].rearrange("p (b n) -> p b n", n=128),
                    AX.X, Alu.add,
                )

            # ---- pool 1: transpose pooled rows back to [e, batch] ----
            pt1 = acc_pool.tile([128, NCH * nb], dt.bfloat16)
            for c in range(NCH):
                eng = nc.scalar if c % 2 else nc.sync
                eng.dma_start_transpose(
                    pt1[:, c * nb:(c + 1) * nb], p_d1[:, c * 128:(c + 1) * 128]
                )

            # ---- pool 2: normalize p = p_raw / sum(E), one multiply ----
            recip = acc_pool.tile([128, nb], dt.float32)
            nc.vector.reciprocal(recip[:], esum2[:])
            p2_bf = acc_pool.tile([128, NCH * nb], dt.bfloat16)
            nc.vector.tensor_tensor(
                p2_bf[:].rearrange("p (c m) -> p c m", c=NCH),
                p2_all[:].rearrange("p (c m) -> p c m", c=NCH),
                recip[:].rearrange("p (one m) -> p one m", one=1)
                    .broadcast_to((128, NCH, nb)),
                Alu.mult,
            )

            # ---- MLP + head ----
            rhs_of = {
                0: lambda ei: pt1[:, ei * nb:(ei + 1) * nb],
                1: lambda ei: p2_bf[:, ei * nb:(ei + 1) * nb],
                2: lambda ei: qv_sb[:, ei * nb:(ei + 1) * nb],
            }
            z_tiles = []
            for inp_i in range(3):
                z_sb = mlp_pool.tile([128, NCH * nb], dt.bfloat16, name=f"z{inp_i}")
                for eo in range(NCH):
                    # reuses the score-phase sp1 banks (free by the MLP phase)
                    pz = psum_pool.tile(
                        [128, nb], dt.float32, tag="sp1", bufs=2, name="pz"
                    )
                    for ei in range(NCH):
                        nc.tensor.matmul(
                            pz[:],
                            w21t_sb[:, (ei * NCH + eo) * 128:(ei * NCH + eo + 1) * 128],
                            rhs_of[inp_i](ei),
                            start=(ei == 0), stop=(ei == NCH - 1),
                        )
                    nc.scalar.activation(
                        z_sb[:, eo * nb:(eo + 1) * nb], pz[:], AF.Relu,
                        bias=b21_sb[:, eo:eo + 1],
                    )
                z_tiles.append(z_sb)
            z1, z2, zq = z_tiles
            d_tiles = []
            for di, (a_t, b_t) in enumerate(((z1, z2), (z1, zq), (z2, zq))):
                d_f = mlp_pool.tile(
                    [128, NCH * nb], dt.float32, tag="d_f", bufs=2, name="d_f"
                )
                nc.vector.tensor_sub(d_f[:], a_t[:], b_t[:])
                d_b = mlp_pool.tile([128, NCH * nb], dt.bfloat16, name=f"d{di}")
                nc.scalar.activation(d_b[:], d_f[:], AF.Abs)
                d_tiles.append(d_b)
            po = psum_pool.tile([1, nb], dt.float32, tag="S2", name="po")
            k = 0
            nmm = 5 * NCH
            for s, zt in enumerate([z1, z2] + d_tiles):
                for c in range(NCH):
                    nc.tensor.matmul(
                        po[0:1, :], w3_sb[:, s * NCH + c:s * NCH + c + 1],
                        zt[:, c * nb:(c + 1) * nb],
                        start=(k == 0), stop=(k == nmm - 1),
                    )
                    k += 1
            o_sb = mlp_pool.tile([1, nb], dt.float32)
            nc.scalar.activation(o_sb[:], po[:], AF.Relu, bias=b3_sb[0:1, 0:1])
            nc.sync.dma_start(out[:], o_sb[:])

    nc.compile()
    return nc


def _get_program(nquad):
    if nquad not in _PROGRAM_CACHE:
        _PROGRAM_CACHE[nquad] = _build_program(nquad)
    return _PROGRAM_CACHE[nquad]


def _prep_core_inputs(Xc):
    """Per-core X slice (mc, 2306, 128) fp32/bf16 -> DMA-ready arrays."""
    mc = Xc.shape[0]
    nquad = mc // 4
    X16 = Xc if Xc.dtype == BF16 else Xc.astype(BF16)
    rows = np.r_[0:2 * EMB, 2 * EMB + 1:3 * EMB + 1]
    xn = X16[:, rows, :]                                   # (mc, 2304, 128)
    xn = xn.reshape(nquad, 4, 18, 128, 128)                # q b c p n
    xn = np.ascontiguousarray(xn.transpose(0, 3, 2, 1, 4)) # q p c b n
    xn = xn.reshape(nquad, 128, 18 * 512)

    xp1 = X16[:, EMB:2 * EMB, :]                           # (mc, 768, 128)
    xp1 = xp1.reshape(nquad, 4, EMB, 128)                  # q b e n
    xpt1 = np.ascontiguousarray(xp1.transpose(0, 3, 1, 2)) # q n b e
    xpt1 = xpt1.reshape(nquad, 128, 4 * EMB)

    qvv = np.ascontiguousarray(X16[:, 0:EMB, 0].T)         # (768, mc)
    qvv = qvv.reshape(NCH, 128, mc)                        # c p m
    qvv = np.ascontiguousarray(qvv.transpose(1, 0, 2)).reshape(128, NCH * mc)
    return xn, xpt1, qvv


def _prep_weights(Wa, va, W1, b1, W2, b2, W3, b3):
    wat = Wa.T.astype(np.float32)                          # (1536, 128)
    wat = wat.reshape(12, 128, 128).transpose(1, 0, 2)     # p c k
    wat = np.ascontiguousarray(wat).reshape(128, 12 * 128).astype(BF16)

    va_rep = np.ascontiguousarray(
        np.repeat(va[0][:, None], 128, axis=1)
    ).astype(BF16)                                         # (128, 128)

    W21 = (W2.astype(np.float32) @ W1.astype(np.float32))
    w21t = W21.T.reshape(NCH, 128, NCH, 128).transpose(1, 0, 2, 3)  # p ei eo c
    w21t = np.ascontiguousarray(w21t).reshape(128, NCH * NCH * 128).astype(BF16)

    w3 = W3[0].astype(np.float32).reshape(5, NCH, 128).transpose(2, 0, 1)  # p s c
    w3 = np.ascontiguousarray(w3).reshape(128, 5 * NCH).astype(BF16)

    b21 = (W2.astype(np.float32) @ b1.astype(np.float32) + b2.astype(np.float32))
    b21 = np.ascontiguousarray(b21.reshape(NCH, 128).T).astype(np.float32)  # (128, 6)
    b3a = np.array(b3, np.float32).reshape(1, 1)
    return dict(wat=wat, va_rep=va_rep, w21t=w21t, w3=w3, b21=b21, b3=b3a)


def kernel(X, Wa, va, W1, b1, W2, b2, W3, b3):
    from concourse.bass_utils import run_bass_kernel_spmd

    X = np.asarray(X)
    w = _prep_weights(
        np.asarray(Wa), np.asarray(va), np.asarray(W1), np.asarray(b1),
        np.asarray(W2), np.asarray(b2), np.asarray(W3), np.asarray(b3),
    )
    mc = X.shape[0] // N_CORES
    in_maps = []
    for c in range(N_CORES):
        xn, xpt1, qvv = _prep_core_inputs(X[c * mc:(c + 1) * mc])
        in_maps.append(dict(xn=xn, xpt1=xpt1, qv=qvv, **w))
    nc = _get_program(mc // 4)
    res = run_bass_kernel_spmd(nc, in_maps, list(range(N_CORES)))
    out = np.concatenate(
        [res.results[i]["out"].reshape(-1) for i in range(N_CORES)]
    )
    return out.astype(np.float32)


if __name__ == "__main__":
    # smoke-build
    nc = _build_program(NQUAD)
    print("program built ok")
